# revision 3
# baseline (speedup 1.0000x reference)
# Trainium2 Bass kernel for nn_BPDecoder: per-sample scaled Gram matrix +
# complementary log-log link.
#
#   rk  = sigmoid(rk_logit)                       [64]
#   X_j = Z_j @ diag(rk) @ Z_j^T                  [2048, 2048] per sample j
#   out = 1 - exp(-exp(min(X, 10)))
#
# Sharding: one sample j per NeuronCore (J=8 == n_cores). Each core
# computes its full [2048, 2048] output slab.
#
# The clamp at 10 is dropped on device: for X >= ~2.81, 1-exp(-exp(X))
# rounds to exactly 1.0 in fp32 (with or without the clamp), and exp(X)
# cannot overflow for the attainable X range, so results are identical.
#
# Host-side work is layout-only: Z is transposed per sample so each core
# DMA-loads Zt [64, 2048] straight into SBUF partitions (d on partitions).
# All arithmetic (sigmoid, rk scaling, matmuls, exp/exp/1-u link) runs on
# device.

import sys

sys.path.insert(0, "/opt/trn_rl_repo")

import numpy as np

J, N, D = 8, 2048, 64
N_CORES = 8

# Set by test harness to capture a profiled run. When True, the run is
# traced (NTFF) and LAST_EXEC_NS is populated with the max per-core
# kernel execution time in ns.
TRACE = False
LAST_EXEC_NS = None

_CACHE = {}


def _build_nc(repeats=1, internal_out=False):
    import concourse.mybir as mybir
    import concourse.tile as tile
    from concourse import bacc

    f32 = mybir.dt.float32
    AF = mybir.ActivationFunctionType
    OP = mybir.AluOpType

    nc = bacc.Bacc("TRN2", target_bir_lowering=False, debug=False,
                   num_devices=N_CORES)
    zt = nc.dram_tensor("zt", [D, N], f32, kind="ExternalInput").ap()
    rkl = nc.dram_tensor("rk_logit", [D, 1], f32, kind="ExternalInput").ap()
    if internal_out:
        # Timing variant: full-size scratch output in internal DRAM so the
        # host fetch (134 MB over the tunnel) drops out of the wall clock.
        dummy = nc.dram_tensor("dummy_out", [1, 1], f32, kind="ExternalOutput").ap()
    else:
        out = nc.dram_tensor("out", [N, N], f32, kind="ExternalOutput").ap()

    NB = N // 128          # 16 row-blocks of 128 output rows
    FT = 512               # matmul free-dim tile (one PSUM bank fp32)
    GROUP = 4              # row-blocks staged per output DMA (4 MiB each)

    with tile.TileContext(nc) as tc:
        with (
            tc.tile_pool(name="const", bufs=1) as cpool,
            tc.tile_pool(name="zpool", bufs=1) as zpool,
            tc.tile_pool(name="psum", bufs=2, space="PSUM") as ppool,
            tc.tile_pool(name="work", bufs=3) as wpool,
            tc.tile_pool(name="obuf", bufs=2) as opool,
            tc.tile_pool(name="dram", bufs=1, space="DRAM") as dpool,
        ):
            if internal_out:
                out = dpool.tile([N, N], f32, tag="scratch_out")

            def body(_iv=None):
                # rk = sigmoid(rk_logit) = 1 / (1 + exp(-l)), on [64, 1]
                rkt = cpool.tile([D, 1], f32, tag="rkl")
                nc.sync.dma_start(out=rkt[:], in_=rkl[:])
                e = cpool.tile([D, 1], f32, tag="e")
                nc.scalar.activation(e[:], rkt[:], AF.Exp, scale=-1.0)
                denom = cpool.tile([D, 1], f32, tag="denom")
                nc.vector.tensor_scalar_add(denom[:], e[:], 1.0)
                rk = cpool.tile([D, 1], f32, tag="rk")
                nc.vector.reciprocal(rk[:], denom[:])

                # Zt [64, 2048] and the rk-scaled copy Wt
                ztile = zpool.tile([D, N], f32, tag="zt")
                nc.sync.dma_start(out=ztile[:], in_=zt[:])
                wt = zpool.tile([D, N], f32, tag="wt")
                nc.vector.tensor_scalar_mul(wt[:], ztile[:], rk[:])

                for g in range(NB // GROUP):
                    ob = opool.tile([128, GROUP, N], f32, tag="ob")
                    for b in range(GROUP):
                        i = g * GROUP + b
                        # X row-block [128, 2048]: lhsT = Wt columns for
                        # these 128 output rows, rhs = all of Zt. K = 64.
                        ps = ppool.tile([128, N], f32, tag="ps")
                        for c in range(N // FT):
                            nc.tensor.matmul(
                                ps[:, c * FT:(c + 1) * FT],
                                lhsT=wt[:, i * 128:(i + 1) * 128],
                                rhs=ztile[:, c * FT:(c + 1) * FT],
                                start=True, stop=True,
                            )
                        t1 = wpool.tile([128, N], f32, tag="t1")
                        nc.scalar.activation(t1[:], ps[:], AF.Exp)
                        u = wpool.tile([128, N], f32, tag="u")
                        nc.scalar.activation(u[:], t1[:], AF.Exp, scale=-1.0)
                        nc.vector.tensor_scalar(
                            ob[:, b, :], u[:], -1.0, 1.0, OP.mult, OP.add
                        )
                    dst = out[g * (128 * GROUP):(g + 1) * (128 * GROUP), :] \
                        .rearrange("(b p) n -> p b n", p=128)
                    nc.sync.dma_start(out=dst, in_=ob[:])

            if repeats == 1:
                body()
            else:
                with tc.For_i(0, repeats, 1):
                    body()

            if internal_out:
                z1 = cpool.tile([1, 1], f32, tag="dummy1")
                nc.vector.memset(z1[:], 0.0)
                nc.sync.dma_start(out=dummy[:], in_=z1[:])

    nc.compile()
    return nc


def kernel(Z, rk_logit):
    global LAST_EXEC_NS
    from concourse.bass_utils import run_bass_kernel_spmd

    Z = np.asarray(Z, dtype=np.float32)
    rk_logit = np.asarray(rk_logit, dtype=np.float32)
    assert Z.shape == (J, N, D), Z.shape

    if "nc" not in _CACHE:
        _CACHE["nc"] = _build_nc()
    nc = _CACHE["nc"]

    rkl_in = np.ascontiguousarray(rk_logit.reshape(D, 1))
    in_maps = [
        {"zt": np.ascontiguousarray(Z[j].T), "rk_logit": rkl_in}
        for j in range(J)
    ]

    if TRACE:
        res = run_bass_kernel_spmd(
            nc, in_maps, core_ids=list(range(N_CORES)),
            trace=True, trace_cores=list(range(N_CORES)),
        )
        LAST_EXEC_NS = res.exec_time_ns
    else:
        res = run_bass_kernel_spmd(nc, in_maps, core_ids=list(range(N_CORES)))

    adj = np.stack([res.results[j]["out"] for j in range(J)], axis=0)

    # Returned alongside, matching the reference tuple (adj_recon, Z, rk).
    rk = (1.0 / (1.0 + np.exp(-rk_logit))).astype(np.float32)
    return adj, Z, rk


# revision 9
# speedup vs baseline: 3.2898x; 3.2898x over previous
# Trainium2 Bass kernel for nn_BPDecoder: per-sample scaled Gram matrix +
# complementary log-log link.
#
#   rk  = sigmoid(rk_logit)                       [64]
#   X_j = Z_j @ diag(rk) @ Z_j^T                  [2048, 2048] per sample j
#   out = 1 - exp(-exp(min(X, 10)))
#
# Sharding: one sample j per NeuronCore (J=8 == n_cores). Each core
# computes its full [2048, 2048] output slab.
#
# Algorithm on each core:
#  - Matmuls run in float32r (TF32-like; plain fp32 matmuls are 4x slower
#    on the PE). X is symmetric per sample, so only the upper-triangle
#    512-wide column tiles are computed (40 of 64 tiles); the strictly
#    lower 128x128 blocks left of each row's first computed tile are
#    produced by PE-transposing the finished mirror-image g-blocks
#    (exact fp32 copies) and DVE-copying them back from PSUM.
#  - The clamp at 10 is dropped: for X >= ~2.81, 1-exp(-exp(X)) rounds to
#    exactly 1.0 in fp32 with or without the clamp, and exp(X) cannot
#    overflow to anything that changes the result.
#  - The whole output slab stays resident in SBUF (16 row-block buffers),
#    each row-block DMAd out (1 MiB, contiguous) as soon as its last
#    writer finishes.
#
# Host-side work is layout-only: Z is transposed per sample so each core
# DMA-loads Zt [64, 2048] straight into SBUF partitions. All arithmetic
# (sigmoid, rk scaling, matmuls, exp/exp/1-u link) runs on device.

import sys

sys.path.insert(0, "/opt/trn_rl_repo")

import numpy as np

J, N, D = 8, 2048, 64
N_CORES = 8

_CACHE = {}


def _build_nc(repeats=1, internal_out=False):
    import concourse.mybir as mybir
    import concourse.tile as tile
    from concourse import bacc
    from concourse.masks import make_identity

    f32 = mybir.dt.float32
    f32r = mybir.dt.float32r
    AF = mybir.ActivationFunctionType
    OP = mybir.AluOpType

    nc = bacc.Bacc("TRN2", target_bir_lowering=False, debug=False,
                   num_devices=N_CORES)
    zt = nc.dram_tensor("zt", [D, N], f32, kind="ExternalInput").ap()
    rkl = nc.dram_tensor("rk_logit", [D, 1], f32, kind="ExternalInput").ap()
    if internal_out:
        # Timing variant: full-size scratch output in internal DRAM so the
        # host fetch (134 MB over the tunnel) drops out of the wall clock.
        dummy = nc.dram_tensor("dummy_out", [1, 1], f32,
                               kind="ExternalOutput").ap()
    else:
        out = nc.dram_tensor("out", [N, N], f32, kind="ExternalOutput").ap()

    NB, FT = N // 128, 512

    with tile.TileContext(nc) as tc:
        with (
            tc.tile_pool(name="const", bufs=1) as cpool,
            tc.tile_pool(name="zpool", bufs=1) as zpool,
            tc.tile_pool(name="psum", bufs=1, space="PSUM") as ppool,
            tc.tile_pool(name="trpsum", bufs=4, space="PSUM") as trpool,
            tc.tile_pool(name="work", bufs=2) as wpool,
            tc.tile_pool(name="obuf", bufs=NB) as opool,
            tc.tile_pool(name="dram", bufs=1, space="DRAM") as dpool,
        ):
            if internal_out:
                out = dpool.tile([N, N], f32, tag="scratch_out")

            identity = cpool.tile([128, 128], f32, tag="ident")
            make_identity(nc, identity[:])

            def body(_iv=None):
                # rk = sigmoid(rk_logit) = 1 / (1 + exp(-l)), on [64, 1]
                rkt = cpool.tile([D, 1], f32, tag="rkl")
                nc.sync.dma_start(out=rkt[:], in_=rkl[:])
                e = cpool.tile([D, 1], f32, tag="e")
                nc.scalar.activation(e[:], rkt[:], AF.Exp, scale=-1.0)
                denom = cpool.tile([D, 1], f32, tag="denom")
                nc.vector.tensor_scalar_add(denom[:], e[:], 1.0)
                rk = cpool.tile([D, 1], f32, tag="rk")
                nc.vector.reciprocal(rk[:], denom[:])

                # Zt [64, 2048]; f32r copies (matmul operands must be
                # produced by instructions with f32r output dtype).
                ztile = zpool.tile([D, N], f32, tag="zt")
                nc.sync.dma_start(out=ztile[:], in_=zt[:])
                zr = zpool.tile([D, N], f32r, tag="zr")
                nc.vector.tensor_copy(zr[:], ztile[:])
                wt = zpool.tile([D, N], f32r, tag="wt")
                nc.vector.tensor_scalar_mul(wt[:], ztile[:], rk[:])

                obs = []
                for i in range(NB):
                    ob = opool.tile([128, N], f32, tag="ob")
                    obs.append(ob)

                for i in range(NB):
                    c0 = i // 4
                    W = N - FT * c0
                    ps = ppool.tile([128, W], f32, tag="ps")
                    for ci, c in enumerate(range(c0, 4)):
                        nc.tensor.matmul(
                            ps[:, ci * FT:(ci + 1) * FT],
                            lhsT=wt[:, i * 128:(i + 1) * 128],
                            rhs=zr[:, c * FT:(c + 1) * FT],
                            start=True, stop=True,
                        )
                    t1 = wpool.tile([128, W], f32, tag="t1")
                    nc.scalar.activation(t1[:], ps[:], AF.Exp)
                    u = wpool.tile([128, W], f32, tag="u")
                    nc.scalar.activation(u[:], t1[:], AF.Exp, scale=-1.0)
                    nc.vector.tensor_scalar(
                        obs[i][:, FT * c0:], u[:], -1.0, 1.0, OP.mult, OP.add
                    )

                    # Mirrors: computed tile (i, c) with 4c > i is the
                    # transpose of output blocks (4c+k, i), k=0..3. PE
                    # transposes (fp32, exact) land in PSUM; DVE copies
                    # them into the target row-block buffers.
                    for c in range(c0, 4):
                        if 4 * c <= i:
                            continue
                        trp = trpool.tile([128, 4, 128], f32, tag="tr")
                        for k in range(4):
                            nc.tensor.transpose(
                                trp[:, k],
                                obs[i][:, c * FT + k * 128:
                                       c * FT + (k + 1) * 128],
                                identity[:],
                            )
                        for k in range(4):
                            nc.vector.tensor_copy(
                                obs[4 * c + k][:, i * 128:(i + 1) * 128],
                                trp[:, k],
                            )

                    # Row-block DMA: all mirror writes into obs[i] were
                    # emitted during earlier rows, so the dependency
                    # tracker orders this correctly.
                    nc.sync.dma_start(out=out[i * 128:(i + 1) * 128, :],
                                      in_=obs[i][:])

            if repeats == 1:
                body()
            else:
                with tc.For_i(0, repeats, 1):
                    body()

            if internal_out:
                z1 = cpool.tile([1, 1], f32, tag="dummy1")
                nc.vector.memset(z1[:], 0.0)
                nc.sync.dma_start(out=dummy[:], in_=z1[:])

    nc.compile()
    return nc


def kernel(Z, rk_logit):
    from concourse.bass_utils import run_bass_kernel_spmd

    Z = np.asarray(Z, dtype=np.float32)
    rk_logit = np.asarray(rk_logit, dtype=np.float32)
    assert Z.shape == (J, N, D), Z.shape

    if "nc" not in _CACHE:
        _CACHE["nc"] = _build_nc()
    nc = _CACHE["nc"]

    rkl_in = np.ascontiguousarray(rk_logit.reshape(D, 1))
    in_maps = [
        {"zt": np.ascontiguousarray(Z[j].T), "rk_logit": rkl_in}
        for j in range(J)
    ]

    res = run_bass_kernel_spmd(nc, in_maps, core_ids=list(range(N_CORES)))
    adj = np.stack([res.results[j]["out"] for j in range(J)], axis=0)

    # Returned alongside, matching the reference tuple (adj_recon, Z, rk).
    rk = (1.0 / (1.0 + np.exp(-rk_logit))).astype(np.float32)
    return adj, Z, rk
